# revision 11
# baseline (speedup 1.0000x reference)
"""Trainium2 Bass kernel for AttentiveTransformer:
   out = sparsemax(GBN(feat @ W.T) * priors)

Data-parallel over 8 NeuronCores: batch 131072 rows -> 8 shards of 16384.
Per core: 32 superchunks x 512 rows; each superchunk = 4 GBN chunks (VBS=128).

Pipeline (per superchunk):
  - load feat rows, PE-transpose -> featT [128k, 512r] (SBUF)
  - per d-slice s: PE matmul x_s = WT_s.T @ featT -> PSUM [128d, 512r]
  - DVE bn_stats (segmented, per GBN chunk) -> mean/var; batched small ops
    compute scale = gamma*rsqrt(var+eps), shift = beta - mean*scale
  - ACT per (s, chunk): xn = x*scale + shift (per-partition scalars)
  - per chunk: PE transpose back -> zT [128r, 512d] PSUM; DVE z = zT * priors
  - sparsemax via hardware top-8: 4x max8 on 128-col segments -> 32 cands,
    max8 -> top8, suppress, max8 -> next8 => sorted top-16 (support <= 15),
    closed-form tau from cumsum (tensor_tensor_scan) + prefix condition,
    ACT final: out = relu(z - tau)
"""
import sys

sys.path.insert(0, "/opt/trn_rl_repo")

import numpy as np
from contextlib import ExitStack

import concourse.bass as bass
import concourse.bacc as bacc
import concourse.tile as tile
from concourse.tile import add_dep_helper
from concourse import mybir
from concourse.bass_utils import run_bass_kernel_spmd

f32 = mybir.dt.float32
i32 = mybir.dt.int32
AF = mybir.ActivationFunctionType
OP = mybir.AluOpType

N_CORES = 8
B, IN, D = 131072, 128, 512
ROWS = B // N_CORES          # 16384 rows per core
SC_ROWS = 512                # superchunk rows (4 GBN chunks)
N_SC = ROWS // SC_ROWS       # 32
VBS = 128
EPS = 1e-5
NEG = -1.0e9


def build_nc():
    nc = bacc.Bacc(None, target_bir_lowering=False)

    priors = nc.dram_tensor("priors", [ROWS, D], f32, kind="ExternalInput")
    feat = nc.dram_tensor("processed_feat", [ROWS, IN], f32, kind="ExternalInput")
    Wd = nc.dram_tensor("W", [D, IN], f32, kind="ExternalInput")
    gam = nc.dram_tensor("gamma", [D], f32, kind="ExternalInput")
    bet = nc.dram_tensor("beta", [D], f32, kind="ExternalInput")
    out = nc.dram_tensor("out", [ROWS, D], f32, kind="ExternalOutput")

    with tile.TileContext(nc) as tc, ExitStack() as ctx:
        singles = ctx.enter_context(tc.tile_pool(name="singles", bufs=1))
        ft_pool = ctx.enter_context(tc.tile_pool(name="ft", bufs=3))
        xn_pool = ctx.enter_context(tc.tile_pool(name="xn", bufs=8))
        p_pool = ctx.enter_context(tc.tile_pool(name="p", bufs=6))
        z_pool = ctx.enter_context(tc.tile_pool(name="z", bufs=6))
        o_pool = ctx.enter_context(tc.tile_pool(name="o", bufs=6))
        st_pool = ctx.enter_context(tc.tile_pool(name="st", bufs=3))
        sm_pool = ctx.enter_context(tc.tile_pool(name="sm", bufs=10))
        ps_ft = ctx.enter_context(tc.tile_pool(name="psft", bufs=2, space="PSUM"))
        ps_x = ctx.enter_context(tc.tile_pool(name="psx", bufs=4, space="PSUM"))
        ps_zt = ctx.enter_context(tc.tile_pool(name="pszt", bufs=2, space="PSUM"))

        # ---------- one-time constants ----------
        ident = singles.tile([128, 128], f32)
        nc.gpsimd.iota(ident, [[1, 128]], base=0, channel_multiplier=-1,
                       allow_small_or_imprecise_dtypes=True)
        nc.vector.tensor_scalar(ident, ident, 0.0, None, OP.is_equal)

        # WT [128k, 512d] resident
        WT = singles.tile([128, D], f32)
        wtp = ps_ft.tile([128, D], f32, tag="ftp")
        for s in range(4):
            wtile = ft_pool.tile([128, 128], f32)
            nc.sync.dma_start(out=wtile, in_=Wd[s * 128:(s + 1) * 128, :])
            nc.tensor.transpose(wtp[:, s * 128:(s + 1) * 128], wtile, ident)
        nc.scalar.copy(WT, wtp)

        # gamma/beta broadcast [128, 4slice, 4chunk]
        gamma44 = singles.tile([128, 4, 4], f32)
        beta44 = singles.tile([128, 4, 4], f32)
        gamma4 = singles.tile([128, 4], f32)
        beta4 = singles.tile([128, 4], f32)
        gr = gam.rearrange("(s p) -> s p", p=128)
        br = bet.rearrange("(s p) -> s p", p=128)
        for s4 in range(4):
            nc.sync.dma_start(out=gamma4[:, s4:s4 + 1],
                              in_=gr[s4].rearrange("(p o) -> p o", o=1))
            nc.sync.dma_start(out=beta4[:, s4:s4 + 1],
                              in_=br[s4].rearrange("(p o) -> p o", o=1))
        for c4 in range(4):
            nc.vector.tensor_copy(gamma44[:, :, c4], gamma4)
            nc.vector.tensor_copy(beta44[:, :, c4], beta4)

        eps_t = singles.tile([128, 1], f32)
        nc.vector.memset(eps_t, EPS)

        rho16 = singles.tile([128, 16], f32)
        nc.gpsimd.iota(rho16, [[1, 16]], base=1, channel_multiplier=0,
                       allow_small_or_imprecise_dtypes=True)
        invrho = singles.tile([128, 16], f32)
        nc.vector.reciprocal(invrho, rho16)

        pri_r = priors.rearrange("(n p) d -> n p d", p=128)
        out_r = out.rearrange("(n p) d -> n p d", p=128)

        # ---------- main loop ----------
        for sc in range(N_SC):
            r0 = sc * SC_ROWS
            # feat -> featT [128k, 512rows]
            ftp = ps_ft.tile([128, SC_ROWS], f32)
            for q in range(4):
                ftile = ft_pool.tile([128, 128], f32)
                nc.sync.dma_start(
                    out=ftile, in_=feat[r0 + q * 128:r0 + (q + 1) * 128, :])
                nc.tensor.transpose(ftp[:, q * 128:(q + 1) * 128], ftile, ident)
            featT = ft_pool.tile([128, SC_ROWS], f32, tag="featT")
            nc.scalar.copy(featT, ftp)

            # GEMM per d-slice + stats
            stats = st_pool.tile([128, 4, 4, 6], f32)
            xps = []
            bn_insts = []
            for s in range(4):
                xp = ps_x.tile([128, SC_ROWS], f32)
                nc.tensor.matmul(xp, WT[:, s * 128:(s + 1) * 128], featT)
                for c in range(4):
                    bi = nc.vector.bn_stats(
                        out=stats[:, s, c],
                        in_=xp[:, c * VBS:(c + 1) * VBS])
                    bn_insts.append(bi)
                xps.append(xp)

            # batched stat math on [128,4,4]
            me = stats[:, :, :, 1]
            mo = stats[:, :, :, 4]
            M2e = stats[:, :, :, 2]
            M2o = stats[:, :, :, 5]
            dm = sm_pool.tile([128, 4, 4], f32, tag="dm")
            m2 = sm_pool.tile([128, 4, 4], f32, tag="m2")
            sm = sm_pool.tile([128, 4, 4], f32, tag="sm")
            sd = sm_pool.tile([128, 4, 4], f32, tag="sd")
            isd = sm_pool.tile([128, 4, 4], f32, tag="isd")
            sscale = sm_pool.tile([128, 4, 4], f32, tag="sscale")
            tshift = sm_pool.tile([128, 4, 4], f32, tag="tshift")
            i1 = nc.vector.tensor_tensor(dm, me, mo, OP.subtract)
            i2 = nc.vector.tensor_tensor(m2, M2e, M2o, OP.add)
            for bi in bn_insts:
                add_dep_helper(i1.ins, bi.ins, sync=True, reason="stats raw")
                add_dep_helper(i2.ins, bi.ins, sync=True, reason="stats raw")
            nc.vector.tensor_tensor(dm, dm, dm, OP.mult)
            # m2 = dm*32 + m2   (full M2 of 128 elems)
            nc.vector.scalar_tensor_tensor(m2, dm, 32.0, m2, OP.mult, OP.add)
            i3 = nc.vector.tensor_tensor(sm, me, mo, OP.add)  # 2*mean
            for bi in bn_insts:
                add_dep_helper(i3.ins, bi.ins, sync=True, reason="stats raw")
            # sd = sqrt(m2/128 + eps)
            nc.scalar.activation(sd, m2, AF.Sqrt, bias=eps_t, scale=1.0 / VBS)
            nc.vector.reciprocal(isd, sd)
            nc.vector.tensor_tensor(sscale, isd, gamma44, OP.mult)
            nc.vector.tensor_tensor(sm, sm, sscale, OP.mult)  # 2*mean*scale
            nc.vector.scalar_tensor_tensor(tshift, sm, -0.5, beta44,
                                           OP.mult, OP.add)

            # normalize: xn = x*scale + shift  (per-partition scalars)
            xns = []
            for s in range(4):
                xn = xn_pool.tile([128, SC_ROWS], f32)
                for c in range(4):
                    nc.scalar.activation(
                        out=xn[:, c * VBS:(c + 1) * VBS],
                        in_=xps[s][:, c * VBS:(c + 1) * VBS],
                        func=AF.Identity,
                        bias=tshift[:, s, c:c + 1],
                        scale=sscale[:, s, c:c + 1],
                    )
                xns.append(xn)

            # per chunk: transpose back, priors, sparsemax
            for c in range(4):
                ci = sc * 4 + c
                ztp = ps_zt.tile([128, D], f32)
                for s in range(4):
                    nc.tensor.transpose(
                        ztp[:, s * 128:(s + 1) * 128],
                        xns[s][:, c * VBS:(c + 1) * VBS], ident)
                ptile = p_pool.tile([128, D], f32)
                nc.sync.dma_start(out=ptile, in_=pri_r[ci])
                z = z_pool.tile([128, D], f32)
                nc.vector.tensor_tensor(z, ztp, ptile, OP.mult)

                # top-16 extraction
                l1 = sm_pool.tile([128, 32], f32, tag="l1")
                for s in range(4):
                    nc.vector.max(out=l1[:, s * 8:(s + 1) * 8],
                                  in_=z[:, s * 128:(s + 1) * 128])
                t16 = sm_pool.tile([128, 16], f32, tag="t16")
                nc.vector.max(out=t16[:, 0:8], in_=l1)
                sup = sm_pool.tile([128, 32], f32, tag="sup")
                nc.vector.tensor_scalar(sup, l1, t16[:, 7:8], NEG,
                                        OP.is_ge, OP.mult)
                nc.vector.tensor_tensor(sup, l1, sup, OP.add)
                nc.vector.max(out=t16[:, 8:16], in_=sup)

                # closed-form tau on sorted top-16
                cs = sm_pool.tile([128, 16], f32, tag="cs")
                nc.vector.tensor_tensor_scan(cs, t16, t16, -1.0, OP.add,
                                             OP.bypass)
                rz = sm_pool.tile([128, 16], f32, tag="rz")
                nc.vector.tensor_tensor(rz, t16, rho16, OP.mult)
                cond = sm_pool.tile([128, 17], f32, tag="cond")
                nc.gpsimd.memset(cond[:, 16:17], 0.0)
                nc.vector.tensor_tensor(cond[:, 0:16], rz, cs, OP.is_gt)
                dcn = sm_pool.tile([128, 16], f32, tag="dcn")
                nc.vector.tensor_tensor(dcn, cond[:, 1:17], cond[:, 0:16],
                                        OP.subtract)  # -delta_{j=k}
                tauj = sm_pool.tile([128, 16], f32, tag="tauj")
                nc.vector.tensor_tensor(tauj, cs, invrho, OP.mult)
                scr = sm_pool.tile([128, 16], f32, tag="scr")
                negtau = sm_pool.tile([128, 1], f32, tag="negtau")
                nc.vector.tensor_tensor(scr, tauj, dcn, OP.mult)
                nc.vector.tensor_reduce(out=negtau, in_=scr,
                                        axis=mybir.AxisListType.X, op=OP.add)

                o_t = o_pool.tile([128, D], f32)
                nc.scalar.activation(o_t, z, AF.Relu, bias=negtau[:, 0:1],
                                     scale=1.0)
                nc.sync.dma_start(out=out_r[ci], in_=o_t)

    nc.compile()
    return nc


_NC_CACHE = None


def kernel(**inputs) -> np.ndarray:
    global _NC_CACHE
    if _NC_CACHE is None:
        _NC_CACHE = build_nc()
    nc = _NC_CACHE

    priors = np.ascontiguousarray(inputs["priors"], dtype=np.float32)
    feat = np.ascontiguousarray(inputs["processed_feat"], dtype=np.float32)
    W = np.ascontiguousarray(inputs["W"], dtype=np.float32)
    gamma = np.ascontiguousarray(inputs["gamma"], dtype=np.float32)
    beta = np.ascontiguousarray(inputs["beta"], dtype=np.float32)

    in_maps = []
    for i in range(N_CORES):
        sl = slice(i * ROWS, (i + 1) * ROWS)
        in_maps.append({
            "priors": priors[sl],
            "processed_feat": feat[sl],
            "W": W,
            "gamma": gamma,
            "beta": beta,
        })
    res = run_bass_kernel_spmd(nc, in_maps, core_ids=list(range(N_CORES)))
    return np.concatenate([r["out"] for r in res.results], axis=0)


if __name__ == "__main__":
    rng = np.random.default_rng(0)
    inputs = {
        "priors": rng.random((B, D), dtype=np.float32),
        "processed_feat": rng.standard_normal((B, IN), dtype=np.float32),
        "W": (rng.standard_normal((D, IN), dtype=np.float32) * 0.1),
        "gamma": np.ones(D, dtype=np.float32),
        "beta": np.zeros(D, dtype=np.float32),
    }
    out = kernel(**inputs)
    print("out", out.shape, out.dtype, float(out.sum()))


# revision 14
# speedup vs baseline: 1.2140x; 1.2140x over previous
"""Trainium2 Bass kernel for AttentiveTransformer:
   out = sparsemax(GBN(feat @ W.T) * priors)

Data-parallel over 8 NeuronCores: batch 131072 rows -> 8 shards of 16384.
Per core: 32 superchunks x 512 rows; each superchunk = 4 GBN chunks (VBS=128).

Pipeline (per superchunk):
  - one batched DMA each for feat/priors/out per superchunk
  - feat -> PE transpose -> featT [128k, 512r] (SBUF)
  - per d-slice s: PE matmul x_s = WT_s.T @ featT -> PSUM [128d, 512r]
  - DVE bn_stats per (slice, chunk) -> even/odd count/mean/M2; gpsimd merges
    and computes scale = gamma*rsqrt(var+eps), shift = beta - mean*scale
  - ACT per (s, chunk): xn = x*scale + shift (per-partition scalars)
  - per chunk: PE transpose back -> zT [128r, 512d] PSUM; DVE z = zT * priors
  - sparsemax via hardware top-8: 4x max8 on 128-col segments -> 32 cands,
    max8 -> top8, suppress, max8 -> next8 => sorted top-16 (support <= 15),
    closed-form tau from cumsum (tensor_tensor_scan) + prefix condition,
    ACT final: out = relu(z - tau)
"""
import sys

sys.path.insert(0, "/opt/trn_rl_repo")

import numpy as np
from contextlib import ExitStack

import concourse.bass as bass
import concourse.bacc as bacc
import concourse.tile as tile
from concourse.tile import add_dep_helper
from concourse import mybir
from concourse.bass_utils import run_bass_kernel_spmd

f32 = mybir.dt.float32
AF = mybir.ActivationFunctionType
OP = mybir.AluOpType

N_CORES = 8
B, IN, D = 131072, 128, 512
ROWS = B // N_CORES          # 16384 rows per core
SC_ROWS = 512                # superchunk rows (4 GBN chunks)
N_SC = ROWS // SC_ROWS       # 32
VBS = 128
EPS = 1e-5
NEG = -1.0e9


def build_nc():
    nc = bacc.Bacc(None, target_bir_lowering=False)

    priors = nc.dram_tensor("priors", [ROWS, D], f32, kind="ExternalInput")
    feat = nc.dram_tensor("processed_feat", [ROWS, IN], f32, kind="ExternalInput")
    Wd = nc.dram_tensor("W", [D, IN], f32, kind="ExternalInput")
    gam = nc.dram_tensor("gamma", [D], f32, kind="ExternalInput")
    bet = nc.dram_tensor("beta", [D], f32, kind="ExternalInput")
    out = nc.dram_tensor("out", [ROWS, D], f32, kind="ExternalOutput")

    with tile.TileContext(nc) as tc, ExitStack() as ctx:
        singles = ctx.enter_context(tc.tile_pool(name="singles", bufs=1))
        ft_pool = ctx.enter_context(tc.tile_pool(name="ft", bufs=3))
        xn_pool = ctx.enter_context(tc.tile_pool(name="xn", bufs=8))
        xs_pool = ctx.enter_context(tc.tile_pool(name="xs", bufs=8))
        p_pool = ctx.enter_context(tc.tile_pool(name="p", bufs=3))
        z_pool = ctx.enter_context(tc.tile_pool(name="z", bufs=6))
        o_pool = ctx.enter_context(tc.tile_pool(name="o", bufs=3))
        st_pool = ctx.enter_context(tc.tile_pool(name="st", bufs=3))
        sm_pool = ctx.enter_context(tc.tile_pool(name="sm", bufs=10))
        ps_ft = ctx.enter_context(tc.tile_pool(name="psft", bufs=2, space="PSUM"))
        ps_x = ctx.enter_context(tc.tile_pool(name="psx", bufs=3, space="PSUM"))
        ps_zt = ctx.enter_context(tc.tile_pool(name="pszt", bufs=3, space="PSUM"))

        # ---------- one-time constants ----------
        ident = singles.tile([128, 128], f32)
        nc.gpsimd.iota(ident, [[1, 128]], base=0, channel_multiplier=-1,
                       allow_small_or_imprecise_dtypes=True)
        nc.vector.tensor_scalar(ident, ident, 0.0, None, OP.is_equal)

        # WT [128k, 512d] resident
        WT = singles.tile([128, D], f32)
        wtp = ps_ft.tile([128, D], f32, tag="ftp")
        for s in range(4):
            wtile = ft_pool.tile([128, 128], f32, tag="wtile")
            nc.sync.dma_start(out=wtile, in_=Wd[s * 128:(s + 1) * 128, :])
            nc.tensor.transpose(wtp[:, s * 128:(s + 1) * 128], wtile, ident)
        nc.scalar.copy(WT, wtp)

        # gamma/beta broadcast [128, 4slice, 4chunk]
        gamma44 = singles.tile([128, 4, 4], f32)
        beta44 = singles.tile([128, 4, 4], f32)
        gamma4 = singles.tile([128, 4], f32)
        beta4 = singles.tile([128, 4], f32)
        gr = gam.rearrange("(s p) -> s p", p=128)
        br = bet.rearrange("(s p) -> s p", p=128)
        for s4 in range(4):
            nc.sync.dma_start(out=gamma4[:, s4:s4 + 1],
                              in_=gr[s4].rearrange("(p o) -> p o", o=1))
            nc.sync.dma_start(out=beta4[:, s4:s4 + 1],
                              in_=br[s4].rearrange("(p o) -> p o", o=1))
        for c4 in range(4):
            nc.vector.tensor_copy(gamma44[:, :, c4], gamma4)
            nc.vector.tensor_copy(beta44[:, :, c4], beta4)

        eps_t = singles.tile([128, 1], f32)
        nc.vector.memset(eps_t, EPS)

        rho16 = singles.tile([128, 16], f32)
        nc.gpsimd.iota(rho16, [[1, 16]], base=1, channel_multiplier=0,
                       allow_small_or_imprecise_dtypes=True)
        invrho = singles.tile([128, 16], f32)
        nc.vector.reciprocal(invrho, rho16)

        fe_r = feat.rearrange("(n c p) k -> n p c k", p=128, c=4)
        pr_r = priors.rearrange("(n c p) d -> n p c d", p=128, c=4)
        out_r = out.rearrange("(n c p) d -> n p c d", p=128, c=4)

        # ---------- main loop ----------
        for sc in range(N_SC):
            # feat -> featT [128k, 512rows]
            f4 = ft_pool.tile([128, 4, 128], f32, tag="f4")
            nc.sync.dma_start(out=f4, in_=fe_r[sc])
            ftp = ps_ft.tile([128, SC_ROWS], f32, tag="ftp")
            for q in range(4):
                nc.tensor.transpose(ftp[:, q * 128:(q + 1) * 128], f4[:, q],
                                    ident)
            featT = ft_pool.tile([128, SC_ROWS], f32, tag="featT")
            nc.scalar.copy(featT, ftp)

            p4 = p_pool.tile([128, 4, D], f32)
            nc.sync.dma_start(out=p4, in_=pr_r[sc])

            # GEMM per d-slice + per-chunk bn_stats
            stats = st_pool.tile([128, 4, 4, 6], f32)
            xps = []
            bn_insts = []
            for s in range(4):
                xp = ps_x.tile([128, SC_ROWS], f32)
                nc.tensor.matmul(xp, WT[:, s * 128:(s + 1) * 128], featT)
                xs = xs_pool.tile([128, SC_ROWS], f32)
                nc.scalar.copy(xs, xp)
                for c in range(4):
                    bi = nc.vector.bn_stats(
                        out=stats[:, s, c],
                        in_=xs[:, c * VBS:(c + 1) * VBS])
                    bn_insts.append(bi)
                xps.append(xs)

            # batched stat math on [128,4,4] (gpsimd; reciprocal on DVE)
            me = stats[:, :, :, 1]
            mo = stats[:, :, :, 4]
            M2e = stats[:, :, :, 2]
            M2o = stats[:, :, :, 5]
            dm = sm_pool.tile([128, 4, 4], f32, tag="dm")
            m2 = sm_pool.tile([128, 4, 4], f32, tag="m2")
            sm = sm_pool.tile([128, 4, 4], f32, tag="sm")
            sd = sm_pool.tile([128, 4, 4], f32, tag="sd")
            isd = sm_pool.tile([128, 4, 4], f32, tag="isd")
            sscale = sm_pool.tile([128, 4, 4], f32, tag="sscale")
            tshift = sm_pool.tile([128, 4, 4], f32, tag="tshift")
            i1 = nc.gpsimd.tensor_tensor(dm, me, mo, OP.subtract)
            i2 = nc.gpsimd.tensor_tensor(m2, M2e, M2o, OP.add)
            i3 = nc.gpsimd.tensor_tensor(sm, me, mo, OP.add)
            for bi in bn_insts:
                add_dep_helper(i1.ins, bi.ins, sync=True, reason="stats raw")
                add_dep_helper(i2.ins, bi.ins, sync=True, reason="stats raw")
                add_dep_helper(i3.ins, bi.ins, sync=True, reason="stats raw")
            nc.gpsimd.tensor_tensor(dm, dm, dm, OP.mult)
            # m2 = dm*32 + m2   (full M2 of 128 elems)
            nc.vector.scalar_tensor_tensor(m2, dm, 32.0, m2, OP.mult, OP.add)
            # sd = sqrt(m2/128 + eps)
            nc.scalar.activation(sd, m2, AF.Sqrt, bias=eps_t, scale=1.0 / VBS)
            nc.vector.reciprocal(isd, sd)
            nc.gpsimd.tensor_tensor(sscale, isd, gamma44, OP.mult)
            nc.gpsimd.tensor_tensor(sm, sm, sscale, OP.mult)  # 2*mean*scale
            nc.vector.scalar_tensor_tensor(tshift, sm, -0.5, beta44,
                                           OP.mult, OP.add)

            # normalize: xn = x*scale + shift  (per-partition scalars)
            xns = []
            for s in range(4):
                xn = xn_pool.tile([128, SC_ROWS], f32)
                for c in range(4):
                    nc.scalar.activation(
                        out=xn[:, c * VBS:(c + 1) * VBS],
                        in_=xps[s][:, c * VBS:(c + 1) * VBS],
                        func=AF.Identity,
                        bias=tshift[:, s, c:c + 1],
                        scale=sscale[:, s, c:c + 1],
                    )
                xns.append(xn)

            o4 = o_pool.tile([128, 4, D], f32)
            # per chunk: transpose back, priors, sparsemax
            for c in range(4):
                ztp = ps_zt.tile([128, D], f32)
                for s in range(4):
                    nc.tensor.transpose(
                        ztp[:, s * 128:(s + 1) * 128],
                        xns[s][:, c * VBS:(c + 1) * VBS], ident)
                z = z_pool.tile([128, D], f32)
                nc.vector.tensor_tensor(z, ztp, p4[:, c], OP.mult)

                # top-16 extraction
                l1 = sm_pool.tile([128, 32], f32, tag="l1")
                for s in range(4):
                    nc.vector.max(out=l1[:, s * 8:(s + 1) * 8],
                                  in_=z[:, s * 128:(s + 1) * 128])
                t16 = sm_pool.tile([128, 16], f32, tag="t16")
                nc.vector.max(out=t16[:, 0:8], in_=l1)
                sup = sm_pool.tile([128, 32], f32, tag="sup")
                nc.vector.tensor_scalar(sup, l1, t16[:, 7:8], NEG,
                                        OP.is_ge, OP.mult)
                nc.gpsimd.tensor_tensor(sup, l1, sup, OP.add)
                nc.vector.max(out=t16[:, 8:16], in_=sup)

                # closed-form tau on sorted top-16
                cs = sm_pool.tile([128, 16], f32, tag="cs")
                nc.vector.tensor_tensor_scan(cs, t16, t16, -1.0, OP.add,
                                             OP.bypass)
                rz = sm_pool.tile([128, 16], f32, tag="rz")
                nc.gpsimd.tensor_tensor(rz, t16, rho16, OP.mult)
                cond = sm_pool.tile([128, 17], f32, tag="cond")
                nc.gpsimd.memset(cond[:, 16:17], 0.0)
                nc.vector.tensor_tensor(cond[:, 0:16], rz, cs, OP.is_gt)
                dcn = sm_pool.tile([128, 16], f32, tag="dcn")
                nc.gpsimd.tensor_tensor(dcn, cond[:, 1:17], cond[:, 0:16],
                                        OP.subtract)  # -delta_{j=k}
                tauj = sm_pool.tile([128, 16], f32, tag="tauj")
                nc.gpsimd.tensor_tensor(tauj, cs, invrho, OP.mult)
                scr = sm_pool.tile([128, 16], f32, tag="scr")
                negtau = sm_pool.tile([128, 1], f32, tag="negtau")
                nc.vector.tensor_tensor(scr, tauj, dcn, OP.mult)
                nc.vector.tensor_reduce(out=negtau, in_=scr,
                                        axis=mybir.AxisListType.X, op=OP.add)

                nc.scalar.activation(o4[:, c], z, AF.Relu,
                                     bias=negtau[:, 0:1], scale=1.0)
            nc.sync.dma_start(out=out_r[sc], in_=o4)

    nc.compile()
    return nc


_NC_CACHE = None


def kernel(**inputs) -> np.ndarray:
    global _NC_CACHE
    if _NC_CACHE is None:
        _NC_CACHE = build_nc()
    nc = _NC_CACHE

    priors = np.ascontiguousarray(inputs["priors"], dtype=np.float32)
    feat = np.ascontiguousarray(inputs["processed_feat"], dtype=np.float32)
    W = np.ascontiguousarray(inputs["W"], dtype=np.float32)
    gamma = np.ascontiguousarray(inputs["gamma"], dtype=np.float32)
    beta = np.ascontiguousarray(inputs["beta"], dtype=np.float32)

    in_maps = []
    for i in range(N_CORES):
        sl = slice(i * ROWS, (i + 1) * ROWS)
        in_maps.append({
            "priors": priors[sl],
            "processed_feat": feat[sl],
            "W": W,
            "gamma": gamma,
            "beta": beta,
        })
    res = run_bass_kernel_spmd(nc, in_maps, core_ids=list(range(N_CORES)))
    return np.concatenate([r["out"] for r in res.results], axis=0)


if __name__ == "__main__":
    rng = np.random.default_rng(0)
    inputs = {
        "priors": rng.random((B, D), dtype=np.float32),
        "processed_feat": rng.standard_normal((B, IN), dtype=np.float32),
        "W": (rng.standard_normal((D, IN), dtype=np.float32) * 0.1),
        "gamma": np.ones(D, dtype=np.float32),
        "beta": np.zeros(D, dtype=np.float32),
    }
    out = kernel(**inputs)
    print("out", out.shape, out.dtype, float(out.sum()))
